# revision 1
# baseline (speedup 1.0000x reference)
"""MoE CouncilLayer kernel for 8x TRN2 NeuronCores (expert-parallel).

Problem (all-expert MoE, B=2, T=1024, C=768, E=32, H=3072):
    gates = softmax(x @ gate_w + gate_b)                     # [N, E]
    h     = gelu(einsum('nc,ech->neh', x, w1) + b1)          # [N, E, H]
    y     = einsum('neh,ehc->nec', h, w2) + b2               # [N, E, C]
    out   = einsum('ne,nec->nc', gates, y)                   # [N, C]

Sharding: expert-parallel, 4 experts per core; x replicated. Each core
computes its 4 experts' gate-weighted partial sum; host adds the 8
partials (the all-reduce is a cheap 6 MB/core host-side sum).

Per-core layout is feature-major (activations stored [feature, token]):
    mm1: psum[h_blk, t] += w1[c_blk, h_blk].T @ xT[c_blk, t]   (fp16)
    hg  = gelu(psum + b1) * gate_bcast                         (fp16)
    mm2: psum[c_blk, t] += w2[h_blk, c_blk].T @ hg[h_blk, t]   (fp16)
         (+ rank-4 matmul b2T.T @ g_localT folded into the same psum
          accumulation, so b2's gate-weighted contribution is free)
b2's gate-weighted term rides the cross-expert accumulation as fused
DVE scalar_tensor_tensor multiply-adds (per-partition b2 scalar x
token-broadcast gate), keeping it off the PE entirely.

Gates are computed on-device (fp16 PE matmuls for logits, fp32 ACT exp,
one fp16 PE ones-matmul per token chunk to sum over the expert
partition axis, DVE reciprocal); the [128, N] per-expert gate broadcast
is a partition-broadcast DMA through a DRAM bounce buffer. Gate columns
are permuted host-side so every core's 4 local experts sit at columns
0..3 (keeps the SPMD program core-agnostic).

Head scheduling exploits PE's in-order execution: the first four
h-blocks of mm1 are traced cc-OUTER across all 8 psum banks so PE
consumes each x chunk as it lands (DMAs sequenced b1 -> interleaved
w1/x-half0 groups -> gate consts -> prefetched second w1 tile ->
x-half1), saturating PE from ~4us. The gate softmax is traced behind
it and drains on ACT/DVE under the matmul stream.

Cost model (InstructionCostModel timeline, one core): ~997.4 us total,
PE busy 987.9 us of which 983.0 us is the irreducible 4608 fp16
[128x128]@[128x512] matmul stream; idle is ~4us head (first-DMA
latency) + ~4.7us tail (drain + out-DMA + barrier) -> 98.6% of the
fp16 PE roofline. The gate logits and softmax denominators are
col-tiled (tile_position 32-column groups, each group in its own psum
bank at partition offset 32*t4) so token-chunk groups run concurrently
on the PE sub-arrays.
"""

import numpy as np

import concourse.tile as tile
from concourse import bacc, mybir
from concourse.bass_utils import run_bass_kernel_spmd

# Problem dims (hardcoded per harness contract)
B, T, C, E, H = 2, 1024, 768, 32, 3072
N = B * T  # 2048 tokens
NCORES = 8
EL = E // NCORES  # 4 local experts
CB = C // 128  # 6 c-blocks
HB = H // 128  # 24 h-blocks
TCG = 2  # token groups (1024 each)
TG = N // TCG  # 1024
TI = TG // 512  # 512-token chunks per group

F16 = mybir.dt.float16
F32 = mybir.dt.float32
AF = mybir.ActivationFunctionType

_CACHED_NC = None


def build_nc(act=AF.Gelu):
    nc = bacc.Bacc(trn_type="TRN2")

    xT16_d = nc.dram_tensor("xT16", [C, N], F16, kind="ExternalInput")
    gw_d = nc.dram_tensor("gw", [C, E], F16, kind="ExternalInput")
    gb_d = nc.dram_tensor("gb", [E, 1], F32, kind="ExternalInput")
    ones_d = nc.dram_tensor("ones32", [E, EL], F16, kind="ExternalInput")
    w1_d = nc.dram_tensor("w1", [EL, C, H], F16, kind="ExternalInput")
    b1_d = nc.dram_tensor("b1", [128, EL, HB], F32, kind="ExternalInput")
    w2_d = nc.dram_tensor("w2", [EL, H, C], F16, kind="ExternalInput")
    b2P_d = nc.dram_tensor("b2P", [128, EL, CB], F32, kind="ExternalInput")
    outT_d = nc.dram_tensor("outT", [C, N], F32, kind="ExternalOutput")

    with tile.TileContext(nc) as tc:
        with (
            tc.tile_pool(name="const", bufs=1) as cp,
            tc.tile_pool(name="stream", bufs=1) as sp,
            tc.tile_pool(name="psum", bufs=1, space="PSUM") as pp,
            tc.tile_pool(name="dram", bufs=1, space="DRAM") as dp,
        ):
            # --- resident tiles ---
            xT16_sb = cp.tile([128, CB, N], F16)
            gw_sb = cp.tile([128, CB, E], F16)
            gb_sb = cp.tile([E, 1], F32)
            ones_sb = cp.tile([E, EL], F16)
            b1_sb = cp.tile([128, EL, HB], F32)
            b2P_sb = cp.tile([128, EL, CB], F32)
            expT_sb = cp.tile([E, N], F32)
            expT16_sb = cp.tile([E, N], F16)
            g_bcast_sb = cp.tile([128, EL, N], F16)
            g_localT_sb = cp.tile([EL, N], F16)

            # DMA issue order = arrival order on the queue; sequence it so
            # the specially-traced first mm1 block (which only needs b1, its
            # w1 tile, and the FIRST-half token columns of each x chunk) can
            # start ~5us in and then stays ahead of the arrival stream. The
            # gate constants ride between the two x halves; b2T (needed at
            # ~140us) goes last.
            w1t_first = sp.tile([128, CB, 512], F16, tag="w1", bufs=3, name="w1t")
            w1f_ap = w1_d[0, :, :].rearrange("(cc p) h -> p cc h", p=128)[:, :, 0:512]
            x_ap = xT16_d[:, :].rearrange("(cc p) t -> p cc t", p=128)
            # first two groups are single-cc so the opening matmuls fire as
            # early as possible; later groups pair up to amortize DMA overhead.
            # b1 (needed by the first gelu ~7us in) rides after the first group.
            ccs = [slice(0, 1), slice(1, 2), slice(2, 4), slice(4, 6)]
            for ci, cs in enumerate(ccs):
                nc.sync.dma_start(w1t_first[:, cs, :], w1f_ap[:, cs, :])
                nc.sync.dma_start(xT16_sb[:, cs, 0:TG], x_ap[:, cs, 0:TG])
                if ci == 0:
                    nc.sync.dma_start(b1_sb, b1_d[:, :, :])
            nc.sync.dma_start(gw_sb, gw_d[:, :].rearrange("(cc p) e -> p cc e", p=128))
            nc.sync.dma_start(gb_sb, gb_d[:, :])
            nc.sync.dma_start(ones_sb, ones_d[:, :])
            # prefetch e0's second w1 tile ahead of the bulk x second half so
            # mm1 hbg1 isn't gated on the 1.5MB transfer in front of it
            w1t_second = sp.tile([128, CB, 512], F16, tag="w1", bufs=3, name="w1t")
            nc.sync.dma_start(
                w1t_second,
                w1_d[0, :, :].rearrange("(cc p) h -> p cc h", p=128)[:, :, 512:1024],
            )
            nc.sync.dma_start(
                xT16_sb[:, :, TG:N],
                xT16_d[:, :].rearrange("(cc p) t -> p cc t", p=128)[:, :, TG:N],
            )
            nc.sync.dma_start(b2P_sb, b2P_d[:, :, :])

            def emit_softmax():
                # gate logits + exp, denominator, reciprocal, local gates;
                # then broadcast across partitions via a DRAM bounce.
                # lg/dn borrow tag-"y" psum slots (mm2 doesn't need them until
                # ~130us in; these drain by ~15us).
                # each token-chunk logit group gets its own psum bank but is
                # written at partition offset 32*t4, issued in t4-pairs per cc
                # so the two matmuls of a pair run concurrently in different
                # 32-column groups of the PE array (2x col-tiling). Each group
                # has its own start/stop: the has_written zero regions are
                # per-partition-range, so the four groups in one bank are
                # independent accumulation groups.
                lgs = [
                    pp.tile([128, 512], F32, tag="y", bufs=4, name="lg")
                    for _ in range(4)
                ]
                for pair in range(2):
                    for cc in range(CB):
                        for t4 in (2 * pair, 2 * pair + 1):
                            nc.tensor.matmul(
                                lgs[t4][32 * t4 : 32 * (t4 + 1), :],
                                gw_sb[:, cc, :],
                                xT16_sb[:, cc, t4 * 512 : (t4 + 1) * 512],
                                start=(cc == 0),
                                stop=(cc == CB - 1),
                                tile_position=(0, 32 * t4),
                            )
                for t4 in range(N // 512):
                    ts = slice(t4 * 512, (t4 + 1) * 512)
                    lgs4 = lgs[t4][32 * t4 : 32 * (t4 + 1), :]
                    nc.scalar.activation(
                        expT_sb[:, ts], lgs4, AF.Exp, bias=gb_sb, scale=1.0
                    )
                    nc.scalar.activation(
                        expT16_sb[:, ts], lgs4, AF.Exp, bias=gb_sb, scale=1.0
                    )
                # denominators in a second pass: by the time PE reaches these,
                # the exps have drained on ACT, so no per-t4 PE stall. fp16
                # rhs makes them 1 cyc/row (rounding averages out over the
                # 32-term sum), and they are col-tiled like the logits so all
                # four run concurrently.
                dns = [
                    pp.tile([128, 512], F32, tag="y", bufs=4, name="dn")
                    for _ in range(4)
                ]
                for t4 in range(N // 512):
                    ts = slice(t4 * 512, (t4 + 1) * 512)
                    nc.tensor.matmul(
                        dns[t4][32 * t4 : 32 * t4 + EL, :],
                        ones_sb[:, :],
                        expT16_sb[:, ts],
                        start=True,
                        stop=True,
                        tile_position=(0, 32 * t4),
                    )
                for t4 in range(N // 512):
                    ts = slice(t4 * 512, (t4 + 1) * 512)
                    rc = sp.tile([EL, 512], F32, tag="recip", bufs=2, name="rc")
                    nc.vector.reciprocal(rc, dns[t4][32 * t4 : 32 * t4 + EL, :])
                    nc.vector.tensor_mul(g_localT_sb[:, ts], expT_sb[0:EL, ts], rc)
                g_dram = dp.tile([EL, N], F16, name="g_dram")
                nc.sync.dma_start(g_dram, g_localT_sb[:, :])
                for j in range(EL):
                    nc.sync.dma_start(
                        g_bcast_sb[:, j, :],
                        g_dram[j : j + 1, :].to_broadcast((128, N)),
                    )

            def emit_gelu(tg, e, hb, hps, hg):
                for ti in range(TI):
                    lts = slice(ti * 512, (ti + 1) * 512)
                    nc.scalar.activation(
                        hg[:, hb, lts],
                        hps[ti],
                        act,
                        bias=b1_sb[:, e, hb : hb + 1],
                        scale=1.0,
                    )

            def emit_scale(tg, e, hb, hg):
                # in-place gate scale; must be traced after the g_bcast DMAs
                # so Tile sees the RAW dependency
                for ti in range(TI):
                    gts = slice(tg * TG + ti * 512, tg * TG + (ti + 1) * 512)
                    lts = slice(ti * 512, (ti + 1) * 512)
                    nc.vector.tensor_mul(
                        hg[:, hb, lts],
                        hg[:, hb, lts],
                        g_bcast_sb[:, e, gts],
                    )

            def emit_gelu_scale(tg, e, hb, hps, hg):
                emit_gelu(tg, e, hb, hps, hg)
                emit_scale(tg, e, hb, hg)

            def emit_mm1_first(hg):
                # first 4 h-blocks of (tg0, e0), traced cc-OUTER across all 8
                # psum banks: PE's in-order stream then consumes each arriving
                # x chunk immediately (8 matmuls = 1.7us per 512KB chunk vs
                # ~1.4us DMA cadence) instead of stalling on the last chunk
                # of an accumulation group.
                hps8 = [
                    [
                        pp.tile(
                            [128, 512],
                            F32,
                            tag=("h" if hbi < 2 else "y"),
                            bufs=4,
                            name="hps",
                        )
                        for _ in range(TI)
                    ]
                    for hbi in range(4)
                ]
                for cc in range(CB):
                    for ti in range(TI):
                        for hbi in range(4):
                            nc.tensor.matmul(
                                hps8[hbi][ti],
                                w1t_first[:, cc, hbi * 128 : (hbi + 1) * 128],
                                xT16_sb[:, cc, ti * 512 : (ti + 1) * 512],
                                start=(cc == 0),
                                stop=(cc == CB - 1),
                            )
                for hbi in range(4):
                    emit_gelu(0, 0, hbi, hps8[hbi], hg)

            def emit_mm1(tg, e, hg, hbg_start=0, gelu_only=False):
                # mm1: h = gelu(w1.T @ xT + b1) * g
                for hbg in range(hbg_start, HB // 4):
                    if tg == 0 and e == 0 and hbg == 1:
                        w1t = w1t_second
                    else:
                        w1t = sp.tile([128, CB, 512], F16, tag="w1", bufs=3, name="w1t")
                        nc.sync.dma_start(
                            w1t,
                            w1_d[e, :, :].rearrange("(cc p) h -> p cc h", p=128)[
                                :, :, hbg * 512 : (hbg + 1) * 512
                            ],
                        )
                    for hbi in range(4):
                        hb = hbg * 4 + hbi
                        hps = [
                            pp.tile([128, 512], F32, tag="h", bufs=4, name="hps")
                            for _ in range(TI)
                        ]
                        for cc in range(CB):
                            for ti in range(TI):
                                gts = slice(
                                    tg * TG + ti * 512, tg * TG + (ti + 1) * 512
                                )
                                nc.tensor.matmul(
                                    hps[ti],
                                    w1t[:, cc, hbi * 128 : (hbi + 1) * 128],
                                    xT16_sb[:, cc, gts],
                                    start=(cc == 0),
                                    stop=(cc == CB - 1),
                                )
                        if gelu_only:
                            emit_gelu(tg, e, hb, hps, hg)
                        else:
                            emit_gelu_scale(tg, e, hb, hps, hg)

            def emit_mm2(tg, e, hg, yac):
                # mm2: y_psum = b2T.T @ g_localT + sum_hb w2.T @ hg
                for cb in range(CB):
                    w2t = sp.tile([128, HB, 128], F16, tag="w2", bufs=3, name="w2t")
                    nc.sync.dma_start(
                        w2t,
                        w2_d[e, :, :].rearrange("(hb p) c -> p hb c", p=128)[
                            :, :, cb * 128 : (cb + 1) * 128
                        ],
                    )
                    yps = [
                        pp.tile([128, 512], F32, tag="y", bufs=4, name="yps")
                        for _ in range(TI)
                    ]
                    for hb in range(HB):
                        for ti in range(TI):
                            lts = slice(ti * 512, (ti + 1) * 512)
                            nc.tensor.matmul(
                                yps[ti],
                                w2t[:, hb, :],
                                hg[:, hb, lts],
                                start=(hb == 0),
                                stop=(hb == HB - 1),
                            )
                    for ti in range(TI):
                        gts = slice(tg * TG + ti * 512, tg * TG + (ti + 1) * 512)
                        lts = slice(ti * 512, (ti + 1) * 512)
                        if e == 0:
                            # yac = g_0*b2_0 + psum, then += g_j*b2_j for the
                            # other local experts: b2's gate-weighted term via
                            # fused DVE multiply-adds (per-partition scalar
                            # b2P x token-broadcast gate), off the PE
                            nc.vector.scalar_tensor_tensor(
                                out=yac[:, cb, lts],
                                in0=g_bcast_sb[:, 0, gts],
                                scalar=b2P_sb[:, 0, cb : cb + 1],
                                in1=yps[ti],
                                op0=mybir.AluOpType.mult,
                                op1=mybir.AluOpType.add,
                            )
                            for j in range(1, EL):
                                nc.vector.scalar_tensor_tensor(
                                    out=yac[:, cb, lts],
                                    in0=g_bcast_sb[:, j, gts],
                                    scalar=b2P_sb[:, j, cb : cb + 1],
                                    in1=yac[:, cb, lts],
                                    op0=mybir.AluOpType.mult,
                                    op1=mybir.AluOpType.add,
                                )
                        else:
                            nc.vector.tensor_add(
                                yac[:, cb, lts], yps[ti], yac[:, cb, lts]
                            )

            # --- main. Trace order = PE order: the special first-hbg block
            # (fills the x-arrival window), then the gate prologue (dense,
            # data all present; its DVE/DMA tail overlaps the next mm1), then
            # the rest of the expert stream.
            for tg in range(TCG):
                hg = sp.tile([128, HB, TG], F16, tag="hg", bufs=1, name="hg")
                yac = sp.tile([128, CB, TG], F32, tag="yacc", bufs=1, name="yac")
                for e in range(EL):
                    if tg == 0 and e == 0:
                        emit_mm1_first(hg)
                        emit_softmax()
                        for hbi in range(4):
                            emit_scale(0, 0, hbi, hg)
                        emit_mm1(tg, e, hg, hbg_start=1)
                    else:
                        emit_mm1(tg, e, hg)
                    emit_mm2(tg, e, hg, yac)
                for cb in range(CB):
                    for ti in range(TI):
                        nc.sync.dma_start(
                            outT_d[
                                cb * 128 : (cb + 1) * 128,
                                tg * TG + ti * 512 : tg * TG + (ti + 1) * 512,
                            ],
                            yac[:, cb, ti * 512 : (ti + 1) * 512],
                        )

    nc.compile()
    return nc


def _get_nc():
    global _CACHED_NC
    if _CACHED_NC is None:
        _CACHED_NC = build_nc()
    return _CACHED_NC


def make_in_maps(x, gate_w, gate_b, w1, b1, w2, b2):
    x = np.asarray(x, np.float32)
    gate_w = np.asarray(gate_w, np.float32)
    gate_b = np.asarray(gate_b, np.float32)
    w1 = np.asarray(w1, np.float32)
    b1 = np.asarray(b1, np.float32)
    w2 = np.asarray(w2, np.float32)
    b2 = np.asarray(b2, np.float32)

    xT16 = np.ascontiguousarray(x.reshape(N, C).T).astype(np.float16)
    w1_16 = w1.astype(np.float16)
    w2_16 = w2.astype(np.float16)

    ones32 = np.ones((E, EL), np.float16)

    in_maps = []
    for i in range(NCORES):
        lo, hi = EL * i, EL * (i + 1)
        perm = list(range(lo, hi)) + [e for e in range(E) if not (lo <= e < hi)]
        in_maps.append(
            {
                "xT16": xT16,
                "gw": np.ascontiguousarray(gate_w[:, perm]).astype(np.float16),
                "gb": np.ascontiguousarray(gate_b[perm]).reshape(E, 1),
                "ones32": ones32,
                "w1": w1_16[lo:hi],
                "b1": np.ascontiguousarray(
                    b1[lo:hi].reshape(EL, HB, 128).transpose(2, 0, 1)
                ),
                "w2": w2_16[lo:hi],
                "b2P": np.ascontiguousarray(
                    b2[lo:hi].reshape(EL, CB, 128).transpose(2, 0, 1)
                ),
            }
        )
    return in_maps


def kernel(x, gate_w, gate_b, w1, b1, w2, b2, _trace=False, _tmpdir=None):
    nc = _get_nc()
    in_maps = make_in_maps(x, gate_w, gate_b, w1, b1, w2, b2)
    res = run_bass_kernel_spmd(
        nc,
        in_maps,
        core_ids=list(range(NCORES)),
        trace=_trace,
        tmpdir=_tmpdir,
    )
    acc = res.results[0]["outT"].astype(np.float64)
    for r in res.results[1:]:
        acc += r["outT"]
    out = acc.T.reshape(B, T, C).astype(np.float32)
    if _trace:
        kernel._last_results = res
    return out



# revision 2
# speedup vs baseline: 1.2615x; 1.2615x over previous
"""MoE CouncilLayer kernel for 8x TRN2 NeuronCores (expert-parallel, fp8).

Problem (all-expert MoE, B=2, T=1024, C=768, E=32, H=3072):
    gates = softmax(x @ gate_w + gate_b)                     # [N, E]
    h     = gelu(einsum('nc,ech->neh', x, w1) + b1)          # [N, E, H]
    y     = einsum('neh,ehc->nec', h, w2) + b2               # [N, E, C]
    out   = einsum('ne,nec->nc', gates, y)                   # [N, C]

Sharding: expert-parallel, 4 experts per core; x replicated. Each core
computes its 4 experts' gate-weighted partial sum; host adds the 8
partials and applies the deferred 1/4096 fixed-point scale.

All matmuls run as fp8e4 (e4m3) DoubleRow ("double-pumped") instructions,
which the PE executes at 0.5 cycles per output column while contracting
two 128-row k-subtiles per instruction. A plain e4m3 quantization costs
~2.6% RMS noise per operand - far over the error budget - so every
matmul uses a 3-slot hi/lo-cross decomposition that cancels first-order
quantization error:

    w ~ w_hi + w_lo,  x ~ x_hi + x_lo   (hi = RTN e4m3 of the pre-scaled
                                         tensor, lo = RTN of the residual
                                         at natural scale)
    w.T x ~ w_hi.T x_hi + w_hi.T x_lo + w_lo.T x_hi     (error O(eps^2))

The residuals of the pre-scaled tensors are O(ulp) ~ O(1) values, i.e.
inside e4m3's normal range, so no per-slot rescale is needed and all
three products accumulate in a single PSUM group. Cost: 1.5 DoubleRow
instructions per 128-k-chunk = 0.75x the fp16 instruction stream as
billed by the cost model (which charges out_free x 0.5 cycles), for an
end-to-end rel-RMS of ~2e-3 (measured host-side on the real inputs).

Per-core layout is feature-major (activations stored [feature, token]):
    mm1: psum[h, t] = sum_slots w1{h,l}[c2,h].T @ x{h,l}[c2,t]   (fp8 DR)
    hg16 = gelu(psum/4096 + b1)                                  (ACT, fp16)
    hg_hi = fp8(8*hg16)            (Pool tensor_scalar)
    hg_lo = fp8(8*hg16 - hg_hi)    (DVE scalar_tensor_tensor)
    mm2: psum[c, t] = sum_slots w2{h,l}[h2,c].T @ hg{h,l}[h2,t]  (fp8 DR)
    yac += g_e * psum  (+ g_e * b2_e*4096 via DVE stt)           (DVE)
Scales: x*16, w1*256 -> mm1 psum = 4096*h; hg*8, w2*512 -> mm2 psum =
4096*y. The 1/4096 is deferred to the host-side partial reduction, so
gates stay unscaled fp16 (no subnormal loss).

Gate softmax runs on-device with the same 3-slot fp8 trick for the
logits (gate noise feeds the output at full weight, so logits need the
accurate path too); denominators via an fp16 ones-matmul over the expert
partition axis, DVE reciprocal, and a partition-broadcast DMA of the
local gates through a DRAM bounce. Gate columns are permuted host-side
so each core's 4 local experts sit at columns 0..3.

Head scheduling: the first six h-blocks of (tg0, e0) are traced
ccp-OUTER across all 8 psum banks so the PE consumes each fp8 x/w chunk
as it lands; the gate softmax trails it and drains on ACT/DVE under the
matmul stream. Cost model timeline: ~744us, PE-bound (fp8 DR stream
3456+3456 instrs x ~106.7ns + gates).
"""

import numpy as np
import ml_dtypes

import concourse.tile as tile
from concourse import bacc, mybir
from concourse.bass_utils import run_bass_kernel_spmd

# Problem dims (hardcoded per harness contract)
B, T, C, E, H = 2, 1024, 768, 32, 3072
N = B * T  # 2048 tokens
NCORES = 8
EL = E // NCORES  # 4 local experts
CB = C // 128  # 6 c-blocks
HB = H // 128  # 24 h-blocks
CP = CB // 2  # 3 c-block pairs (DoubleRow k-pairs)
HP = HB // 2  # 12 h-block pairs
TCG = 2  # token groups (1024 each)
TG = N // TCG  # 1024
TI = TG // 512  # 512-token chunks per group

SX, SW1, SH, SW2 = 16.0, 256.0, 8.0, 512.0
PSC1 = SX * SW1  # mm1 psum scale (4096)
PSC2 = SH * SW2  # mm2 psum scale (4096), removed host-side

F8 = mybir.dt.float8e4
F16 = mybir.dt.float16
F32 = mybir.dt.float32
DR = mybir.MatmulPerfMode.DoubleRow
AF = mybir.ActivationFunctionType
ALU = mybir.AluOpType
NP8 = ml_dtypes.float8_e4m3

_CACHED_NC = None


def build_nc(act=AF.Gelu):
    nc = bacc.Bacc(trn_type="TRN2")

    xh_d = nc.dram_tensor("xh", [C, N], F8, kind="ExternalInput")
    xl_d = nc.dram_tensor("xl", [C, N], F8, kind="ExternalInput")
    gwh_d = nc.dram_tensor("gwh", [C, E], F8, kind="ExternalInput")
    gwl_d = nc.dram_tensor("gwl", [C, E], F8, kind="ExternalInput")
    gb_d = nc.dram_tensor("gb", [E, 1], F32, kind="ExternalInput")
    ones_d = nc.dram_tensor("ones32", [E, EL], F16, kind="ExternalInput")
    w1h_d = nc.dram_tensor("w1h", [EL, C, H], F8, kind="ExternalInput")
    w1l_d = nc.dram_tensor("w1l", [EL, C, H], F8, kind="ExternalInput")
    b1_d = nc.dram_tensor("b1", [128, EL, HB], F32, kind="ExternalInput")
    w2h_d = nc.dram_tensor("w2h", [EL, H, C], F8, kind="ExternalInput")
    w2l_d = nc.dram_tensor("w2l", [EL, H, C], F8, kind="ExternalInput")
    b2P_d = nc.dram_tensor("b2P", [128, EL, CB], F32, kind="ExternalInput")
    outT_d = nc.dram_tensor("outT", [C, N], F32, kind="ExternalOutput")

    def w1ap(dram, e, hbg):
        return dram[e, :, :].rearrange("(cc p) h -> p cc h", p=128)[
            :, :, hbg * 512 : (hbg + 1) * 512
        ]

    def w2ap(dram, e, cb):
        return dram[e, :, :].rearrange("(hb p) c -> p hb c", p=128)[
            :, :, cb * 128 : (cb + 1) * 128
        ]

    with tile.TileContext(nc) as tc:
        with (
            tc.tile_pool(name="const", bufs=1) as cp,
            tc.tile_pool(name="stream", bufs=1) as sp,
            tc.tile_pool(name="psum", bufs=1, space="PSUM") as pp,
            tc.tile_pool(name="dram", bufs=1, space="DRAM") as dp,
        ):
            # --- resident tiles ---
            xh_sb = cp.tile([128, CB, N], F8)
            xl_sb = cp.tile([128, CB, N], F8)
            gwh_sb = cp.tile([128, CB, E], F8)
            gwl_sb = cp.tile([128, CB, E], F8)
            gb_sb = cp.tile([E, 1], F32)
            ones_sb = cp.tile([E, EL], F16)
            b1_sb = cp.tile([128, EL, HB], F32)
            b2P_sb = cp.tile([128, EL, CB], F32)
            expT_sb = cp.tile([E, N], F32)
            expT16_sb = cp.tile([E, N], F16)
            g_bcast_sb = cp.tile([128, EL, N], F16)
            g_localT_sb = cp.tile([EL, N], F16)

            # DMA issue order = arrival order. The specially-traced first mm1
            # block needs, per cc-pair group: w1h slice -> xh half -> w1l
            # slice -> xl half (the PE's slot order is M, C1, C2 per ccp, so
            # xl arrives third). b1 (first gelu ~8us in) after the first
            # group; gate consts + the hbg1 w1 prefetch ride before the bulk
            # x second halves.
            xh_ap = xh_d[:, :].rearrange("(cc p) t -> p cc t", p=128)
            xl_ap = xl_d[:, :].rearrange("(cc p) t -> p cc t", p=128)
            w1h_first = sp.tile([128, CB, 512], F8, tag="w1", bufs=4, name="w1t")
            w1l_first = sp.tile([128, CB, 512], F8, tag="w1", bufs=4, name="w1t")
            for ccp in range(CP):
                cs = slice(2 * ccp, 2 * ccp + 2)
                nc.sync.dma_start(w1h_first[:, cs, :], w1ap(w1h_d, 0, 0)[:, cs, :])
                nc.sync.dma_start(xh_sb[:, cs, 0:TG], xh_ap[:, cs, 0:TG])
                nc.sync.dma_start(w1l_first[:, cs, :], w1ap(w1l_d, 0, 0)[:, cs, :])
                nc.sync.dma_start(xl_sb[:, cs, 0:TG], xl_ap[:, cs, 0:TG])
                if ccp == 0:
                    nc.sync.dma_start(b1_sb, b1_d[:, :, :])
            nc.sync.dma_start(gwh_sb, gwh_d[:, :].rearrange("(cc p) e -> p cc e", p=128))
            nc.sync.dma_start(gwl_sb, gwl_d[:, :].rearrange("(cc p) e -> p cc e", p=128))
            nc.sync.dma_start(gb_sb, gb_d[:, :])
            nc.sync.dma_start(ones_sb, ones_d[:, :])
            # prefetch e0's hbg1 w1 tiles so mm1 isn't gated on the bulk
            # second-half x transfers in front of it
            w1h_second = sp.tile([128, CB, 512], F8, tag="w1", bufs=4, name="w1t")
            w1l_second = sp.tile([128, CB, 512], F8, tag="w1", bufs=4, name="w1t")
            nc.sync.dma_start(w1h_second, w1ap(w1h_d, 0, 1))
            nc.sync.dma_start(w1l_second, w1ap(w1l_d, 0, 1))
            nc.sync.dma_start(xh_sb[:, :, TG:N], xh_ap[:, :, TG:N])
            nc.sync.dma_start(xl_sb[:, :, TG:N], xl_ap[:, :, TG:N])
            nc.sync.dma_start(b2P_sb, b2P_d[:, :, :])

            def dr3(ps, wh, wl, xh, xl, ccp, nccp):
                # one cc-pair's three hi/lo-cross slots into psum `ps`
                first = ccp == 0
                last = ccp == nccp - 1
                nc.tensor.matmul(ps, wh, xh, start=first, stop=False, perf_mode=DR)
                nc.tensor.matmul(ps, wh, xl, start=False, stop=False, perf_mode=DR)
                nc.tensor.matmul(ps, wl, xh, start=False, stop=last, perf_mode=DR)

            def emit_softmax():
                # gate logits via the same 3-slot fp8 path (gate noise feeds
                # the output at full weight); exp on ACT with the 1/4096
                # psum scale folded in; fp16 ones-matmul denominators; DVE
                # reciprocal; DRAM-bounce partition broadcast.
                # lg/dn borrow tag-"y" psum slots (mm2 needs them ~95us in;
                # these drain by ~20us).
                lgs = [
                    pp.tile([128, 512], F32, tag="y", bufs=4, name="lg")
                    for _ in range(4)
                ]
                for t4 in range(N // 512):
                    ts = slice(t4 * 512, (t4 + 1) * 512)
                    for ccp in range(CP):
                        cs = slice(2 * ccp, 2 * ccp + 2)
                        dr3(
                            lgs[t4][0:E, :],
                            gwh_sb[:, cs, :],
                            gwl_sb[:, cs, :],
                            xh_sb[:, cs, ts],
                            xl_sb[:, cs, ts],
                            ccp,
                            CP,
                        )
                for t4 in range(N // 512):
                    ts = slice(t4 * 512, (t4 + 1) * 512)
                    nc.scalar.activation(
                        expT_sb[:, ts], lgs[t4][0:E, :], AF.Exp, bias=gb_sb,
                        scale=1.0 / PSC1,
                    )
                    nc.scalar.activation(
                        expT16_sb[:, ts], lgs[t4][0:E, :], AF.Exp, bias=gb_sb,
                        scale=1.0 / PSC1,
                    )
                dns = [
                    pp.tile([128, 512], F32, tag="y", bufs=4, name="dn")
                    for _ in range(4)
                ]
                for t4 in range(N // 512):
                    ts = slice(t4 * 512, (t4 + 1) * 512)
                    nc.tensor.matmul(
                        dns[t4][0:EL, :], ones_sb[:, :], expT16_sb[:, ts],
                        start=True, stop=True,
                    )
                for t4 in range(N // 512):
                    ts = slice(t4 * 512, (t4 + 1) * 512)
                    rc = sp.tile([EL, 512], F32, tag="recip", bufs=2, name="rc")
                    nc.vector.reciprocal(rc, dns[t4][0:EL, :])
                    nc.vector.tensor_mul(g_localT_sb[:, ts], expT_sb[0:EL, ts], rc)
                g_dram = dp.tile([EL, N], F16, name="g_dram")
                nc.sync.dma_start(g_dram, g_localT_sb[:, :])
                for j in range(EL):
                    nc.sync.dma_start(
                        g_bcast_sb[:, j, :],
                        g_dram[j : j + 1, :].to_broadcast((128, N)),
                    )

            def emit_gelu(e, hbg, hps, hg16):
                # psum -> fp16 gelu output, one ACT op per (hbi, ti)
                for hbi in range(4):
                    hb = hbg * 4 + hbi
                    for ti in range(TI):
                        lts = slice(ti * 512, (ti + 1) * 512)
                        nc.scalar.activation(
                            hg16[:, hbi, lts],
                            hps[hbi][ti],
                            act,
                            bias=b1_sb[:, e, hb : hb + 1],
                            scale=1.0 / PSC1,
                        )

            def emit_hilo(hbg, hg16, hgh, hgl):
                # hg_hi = fp8(8*hg16) on Pool; hg_lo = fp8(8*hg16 - hg_hi)
                # on DVE (one bulk op each per 4-hb group)
                hs = slice(hbg * 4, (hbg + 1) * 4)
                nc.gpsimd.tensor_scalar_mul(hgh[:, hs, :], hg16[:, :, :], SH)
                nc.vector.scalar_tensor_tensor(
                    out=hgl[:, hs, :],
                    in0=hg16[:, :, :],
                    scalar=SH,
                    in1=hgh[:, hs, :],
                    op0=ALU.mult,
                    op1=ALU.subtract,
                )

            def emit_mm1_first(hgh, hgl):
                # first h-block-group of (tg0, e0), traced ccp-OUTER across
                # all 8 psum banks: PE consumes each arriving fp8 chunk
                # immediately instead of stalling on the tail of one
                # accumulation group.
                hps8 = [
                    [
                        pp.tile(
                            [128, 512], F32,
                            tag=("h" if hbi < 2 else "y"),
                            bufs=4, name="hps",
                        )
                        for _ in range(TI)
                    ]
                    for hbi in range(4)
                ]
                for ccp in range(CP):
                    cs = slice(2 * ccp, 2 * ccp + 2)
                    for si, (wt, xt) in enumerate(
                        [(w1h_first, xh_sb), (w1h_first, xl_sb), (w1l_first, xh_sb)]
                    ):
                        for hbi in range(4):
                            for ti in range(TI):
                                nc.tensor.matmul(
                                    hps8[hbi][ti],
                                    wt[:, cs, hbi * 128 : (hbi + 1) * 128],
                                    xt[:, cs, ti * 512 : (ti + 1) * 512],
                                    start=(ccp == 0 and si == 0),
                                    stop=(ccp == CP - 1 and si == 2),
                                    perf_mode=DR,
                                )
                hg16 = sp.tile([128, 4, TG], F16, tag="hg16", bufs=2, name="hg16")
                emit_gelu(0, 0, hps8, hg16)
                emit_hilo(0, hg16, hgh, hgl)

            def emit_mm1(tg, e, hgh, hgl, hbg_start=0):
                for hbg in range(hbg_start, HB // 4):
                    if tg == 0 and e == 0 and hbg == 1:
                        w1h_t, w1l_t = w1h_second, w1l_second
                    else:
                        w1h_t = sp.tile([128, CB, 512], F8, tag="w1", bufs=4, name="w1t")
                        w1l_t = sp.tile([128, CB, 512], F8, tag="w1", bufs=4, name="w1t")
                        nc.sync.dma_start(w1h_t, w1ap(w1h_d, e, hbg))
                        nc.sync.dma_start(w1l_t, w1ap(w1l_d, e, hbg))
                    hps = [
                        [
                            pp.tile([128, 512], F32, tag="h", bufs=4, name="hps")
                            for _ in range(TI)
                        ]
                        for _ in range(4)
                    ]
                    for hbi in range(4):
                        for ti in range(TI):
                            gts = slice(tg * TG + ti * 512, tg * TG + (ti + 1) * 512)
                            for ccp in range(CP):
                                cs = slice(2 * ccp, 2 * ccp + 2)
                                dr3(
                                    hps[hbi][ti],
                                    w1h_t[:, cs, hbi * 128 : (hbi + 1) * 128],
                                    w1l_t[:, cs, hbi * 128 : (hbi + 1) * 128],
                                    xh_sb[:, cs, gts],
                                    xl_sb[:, cs, gts],
                                    ccp,
                                    CP,
                                )
                    hg16 = sp.tile([128, 4, TG], F16, tag="hg16", bufs=2, name="hg16")
                    emit_gelu(e, hbg, hps, hg16)
                    emit_hilo(hbg, hg16, hgh, hgl)

            def emit_mm2(tg, e, hgh, hgl, yac):
                # psum[c,t] accumulates 12 h-pairs x 3 slots; the gate weight
                # and the (pre-scaled) b2 term apply on DVE into yac.
                for cb in range(CB):
                    w2h_t = sp.tile([128, HB, 128], F8, tag="w2", bufs=4, name="w2t")
                    w2l_t = sp.tile([128, HB, 128], F8, tag="w2", bufs=4, name="w2t")
                    nc.sync.dma_start(w2h_t, w2ap(w2h_d, e, cb))
                    nc.sync.dma_start(w2l_t, w2ap(w2l_d, e, cb))
                    for ti in range(TI):
                        lts = slice(ti * 512, (ti + 1) * 512)
                        gts = slice(tg * TG + ti * 512, tg * TG + (ti + 1) * 512)
                        yps = pp.tile([128, 512], F32, tag="y", bufs=4, name="yps")
                        for hp in range(HP):
                            hs = slice(2 * hp, 2 * hp + 2)
                            first = hp == 0
                            last = hp == HP - 1
                            nc.tensor.matmul(
                                yps, w2h_t[:, hs, :], hgh[:, hs, lts],
                                start=first, stop=False, perf_mode=DR,
                            )
                            nc.tensor.matmul(
                                yps, w2h_t[:, hs, :], hgl[:, hs, lts],
                                start=False, stop=False, perf_mode=DR,
                            )
                            nc.tensor.matmul(
                                yps, w2l_t[:, hs, :], hgh[:, hs, lts],
                                start=False, stop=last, perf_mode=DR,
                            )
                        if e == 0:
                            nc.vector.tensor_mul(
                                yac[:, cb, lts], g_bcast_sb[:, 0, gts], yps
                            )
                            for j in range(EL):
                                # b2 gate-weighted term (b2 pre-scaled x4096)
                                nc.vector.scalar_tensor_tensor(
                                    out=yac[:, cb, lts],
                                    in0=g_bcast_sb[:, j, gts],
                                    scalar=b2P_sb[:, j, cb : cb + 1],
                                    in1=yac[:, cb, lts],
                                    op0=ALU.mult,
                                    op1=ALU.add,
                                )
                        else:
                            ytmp = sp.tile([128, 512], F32, tag="ytmp", bufs=2,
                                           name="ytmp")
                            nc.vector.tensor_mul(
                                ytmp, g_bcast_sb[:, e, gts], yps
                            )
                            nc.vector.tensor_add(
                                yac[:, cb, lts], ytmp, yac[:, cb, lts]
                            )

            # --- main. Trace order = PE order: the special first block
            # (fills the fp8 x/w arrival window), the gate prologue (drains
            # on ACT/DVE under the matmul stream), then the expert stream.
            for tg in range(TCG):
                hgh = sp.tile([128, HB, TG], F8, tag="hgh", bufs=1, name="hgh")
                hgl = sp.tile([128, HB, TG], F8, tag="hgl", bufs=1, name="hgl")
                yac = sp.tile([128, CB, TG], F32, tag="yacc", bufs=1, name="yac")
                for e in range(EL):
                    if tg == 0 and e == 0:
                        emit_mm1_first(hgh, hgl)
                        emit_softmax()
                        emit_mm1(tg, e, hgh, hgl, hbg_start=1)
                    else:
                        emit_mm1(tg, e, hgh, hgl)
                    emit_mm2(tg, e, hgh, hgl, yac)
                for cb in range(CB):
                    for ti in range(TI):
                        nc.sync.dma_start(
                            outT_d[
                                cb * 128 : (cb + 1) * 128,
                                tg * TG + ti * 512 : tg * TG + (ti + 1) * 512,
                            ],
                            yac[:, cb, ti * 512 : (ti + 1) * 512],
                        )

    nc.compile()
    return nc


def _get_nc():
    global _CACHED_NC
    if _CACHED_NC is None:
        _CACHED_NC = build_nc()
    return _CACHED_NC


def _hilo(a, scale):
    hi = (a * scale).astype(NP8)
    lo = (a * scale - hi.astype(np.float32)).astype(NP8)
    return hi, lo


def make_in_maps(x, gate_w, gate_b, w1, b1, w2, b2):
    x = np.asarray(x, np.float32)
    gate_w = np.asarray(gate_w, np.float32)
    gate_b = np.asarray(gate_b, np.float32)
    w1 = np.asarray(w1, np.float32)
    b1 = np.asarray(b1, np.float32)
    w2 = np.asarray(w2, np.float32)
    b2 = np.asarray(b2, np.float32)

    xT = np.ascontiguousarray(x.reshape(N, C).T)
    xh, xl = _hilo(xT, SX)
    w1h, w1l = _hilo(w1, SW1)
    w2h, w2l = _hilo(w2, SW2)

    ones32 = np.ones((E, EL), np.float16)

    in_maps = []
    for i in range(NCORES):
        lo_, hi_ = EL * i, EL * (i + 1)
        perm = list(range(lo_, hi_)) + [e for e in range(E) if not (lo_ <= e < hi_)]
        gwp = np.ascontiguousarray(gate_w[:, perm])
        gwh, gwl = _hilo(gwp, SW1)
        in_maps.append(
            {
                "xh": xh,
                "xl": xl,
                "gwh": gwh,
                "gwl": gwl,
                "gb": np.ascontiguousarray(gate_b[perm]).reshape(E, 1),
                "ones32": ones32,
                "w1h": w1h[lo_:hi_],
                "w1l": w1l[lo_:hi_],
                "b1": np.ascontiguousarray(
                    b1[lo_:hi_].reshape(EL, HB, 128).transpose(2, 0, 1)
                ),
                "w2h": w2h[lo_:hi_],
                "w2l": w2l[lo_:hi_],
                "b2P": np.ascontiguousarray(
                    b2[lo_:hi_].reshape(EL, CB, 128).transpose(2, 0, 1)
                )
                * PSC2,
            }
        )
    return in_maps


def kernel(x, gate_w, gate_b, w1, b1, w2, b2, _trace=False, _tmpdir=None):
    nc = _get_nc()
    in_maps = make_in_maps(x, gate_w, gate_b, w1, b1, w2, b2)
    res = run_bass_kernel_spmd(
        nc,
        in_maps,
        core_ids=list(range(NCORES)),
        trace=_trace,
        tmpdir=_tmpdir,
    )
    acc = res.results[0]["outT"].astype(np.float64)
    for r in res.results[1:]:
        acc += r["outT"]
    out = (acc / PSC2).T.reshape(B, T, C).astype(np.float32)
    if _trace:
        kernel._last_results = res
    return out


# revision 8
# speedup vs baseline: 1.3169x; 1.0439x over previous
"""MoE CouncilLayer kernel for 8x TRN2 NeuronCores (expert-parallel, fp8).

Problem (all-expert MoE, B=2, T=1024, C=768, E=32, H=3072):
    gates = softmax(x @ gate_w + gate_b)                     # [N, E]
    h     = gelu(einsum('nc,ech->neh', x, w1) + b1)          # [N, E, H]
    y     = einsum('neh,ehc->nec', h, w2) + b2               # [N, E, C]
    out   = einsum('ne,nec->nc', gates, y)                   # [N, C]

Sharding: expert-parallel, 4 experts per core; x replicated. Each core
computes its 4 experts' gate-weighted partial sum; host adds the 8
partials and applies the deferred 1/4096 fixed-point scale.

All matmuls run as fp8e4 (e4m3) DoubleRow ("double-pumped") instructions,
which the PE executes at 0.5 cycles per output column while contracting
two 128-row k-subtiles per instruction. A plain e4m3 quantization costs
~2.6% RMS noise per operand - far over the error budget - so every
matmul uses a 3-slot hi/lo-cross decomposition that cancels first-order
quantization error:

    w ~ w_hi + w_lo,  x ~ x_hi + x_lo   (hi = RTN e4m3 of the pre-scaled
                                         tensor, lo = RTN of the residual
                                         at natural scale)
    w.T x ~ w_hi.T x_hi + w_hi.T x_lo + w_lo.T x_hi     (error O(eps^2))

The residuals of the pre-scaled tensors are O(ulp) ~ O(1) values, i.e.
inside e4m3's normal range, so no per-slot rescale is needed and all
three products accumulate in a single PSUM group. Cost: 1.5 DoubleRow
instructions per 128-k-chunk = 0.75x the fp16 instruction stream as
billed by the cost model (which charges out_free x 0.5 cycles), for an
end-to-end rel-RMS of ~2e-3 (measured host-side on the real inputs).

Per-core layout is feature-major (activations stored [feature, token]):
    mm1: psum[h, t] = sum_slots w1{h,l}[c2,h].T @ x{h,l}[c2,t]   (fp8 DR)
    hg16 = gelu(psum/4096 + b1)                                  (ACT, fp16)
    hg_hi = fp8(8*hg16)            (Pool tensor_scalar)
    hg_lo = fp8(8*hg16 - hg_hi)    (DVE scalar_tensor_tensor)
    mm2: psum[c, t] = sum_slots w2{h,l}[h2,c].T @ hg{h,l}[h2,t]  (fp8 DR)
    yac += g_e * psum  (+ g_e * b2_e*4096 via DVE stt)           (DVE)
Scales: x*16, w1*256 -> mm1 psum = 4096*h; hg*8, w2*512 -> mm2 psum =
4096*y. The 1/4096 is deferred to the host-side partial reduction, so
gates stay unscaled fp16 (no subnormal loss).

Gate softmax runs on-device with the same 3-slot fp8 trick for the
logits (gate noise feeds the output at full weight, so logits need the
accurate path too); denominators via an fp16 ones-matmul over the expert
partition axis, DVE reciprocal, and a partition-broadcast DMA of the
local gates through a DRAM bounce. Gate columns are permuted host-side
so each core's 4 local experts sit at columns 0..3.

Head scheduling: the first six h-blocks of (tg0, e0) are traced
ccp-OUTER across all 8 psum banks so the PE consumes each fp8 x/w chunk
as it lands; the gate softmax trails it and drains on ACT/DVE under the
matmul stream. Cost model timeline: ~744us, PE-bound (fp8 DR stream
3456+3456 instrs x ~106.7ns + gates).
"""

import numpy as np
import ml_dtypes

import concourse.tile as tile
from concourse import bacc, mybir
from concourse.bass_utils import run_bass_kernel_spmd

# Problem dims (hardcoded per harness contract)
B, T, C, E, H = 2, 1024, 768, 32, 3072
N = B * T  # 2048 tokens
NCORES = 8
EL = E // NCORES  # 4 local experts
CB = C // 128  # 6 c-blocks
HB = H // 128  # 24 h-blocks
CP = CB // 2  # 3 c-block pairs (DoubleRow k-pairs)
HP = HB // 2  # 12 h-block pairs
TCG = 2  # token groups (1024 each)
TG = N // TCG  # 1024
TI = TG // 512  # 512-token chunks per group

SX, SW1, SH, SW2 = 16.0, 256.0, 8.0, 512.0
PSC1 = SX * SW1  # mm1 psum scale (4096)
PSC2 = SH * SW2  # mm2 psum scale (4096), removed host-side

F8 = mybir.dt.float8e4
F16 = mybir.dt.float16
F32 = mybir.dt.float32
DR = mybir.MatmulPerfMode.DoubleRow
AF = mybir.ActivationFunctionType
ALU = mybir.AluOpType
NP8 = ml_dtypes.float8_e4m3

_CACHED_NC = None


def build_nc(act=AF.Gelu):
    nc = bacc.Bacc(trn_type="TRN2")

    xh_d = nc.dram_tensor("xh", [C, N], F8, kind="ExternalInput")
    xl_d = nc.dram_tensor("xl", [C, N], F8, kind="ExternalInput")
    gwh_d = nc.dram_tensor("gwh", [C, E], F8, kind="ExternalInput")
    gwl_d = nc.dram_tensor("gwl", [C, E], F8, kind="ExternalInput")
    gb_d = nc.dram_tensor("gb", [E, 1], F32, kind="ExternalInput")
    ones_d = nc.dram_tensor("ones32", [E, EL], F16, kind="ExternalInput")
    w1h_d = nc.dram_tensor("w1h", [EL, C, H], F8, kind="ExternalInput")
    w1l_d = nc.dram_tensor("w1l", [EL, C, H], F8, kind="ExternalInput")
    b1_d = nc.dram_tensor("b1", [128, EL, HB], F32, kind="ExternalInput")
    w2h_d = nc.dram_tensor("w2h", [EL, H, C], F8, kind="ExternalInput")
    w2l_d = nc.dram_tensor("w2l", [EL, H, C], F8, kind="ExternalInput")
    b2P_d = nc.dram_tensor("b2P", [128, EL, CB], F32, kind="ExternalInput")
    outT_d = nc.dram_tensor("outT", [C, N], F32, kind="ExternalOutput")

    def w1ap(dram, e, hbg):
        return dram[e, :, :].rearrange("(cc p) h -> p cc h", p=128)[
            :, :, hbg * 512 : (hbg + 1) * 512
        ]

    def w2ap(dram, e, cb):
        return dram[e, :, :].rearrange("(hb p) c -> p hb c", p=128)[
            :, :, cb * 128 : (cb + 1) * 128
        ]

    with tile.TileContext(nc) as tc:
        with (
            tc.tile_pool(name="const", bufs=1) as cp,
            tc.tile_pool(name="stream", bufs=1) as sp,
            tc.tile_pool(name="psum", bufs=1, space="PSUM") as pp,
            tc.tile_pool(name="dram", bufs=1, space="DRAM") as dp,
        ):
            # --- resident tiles ---
            xh_sb = cp.tile([128, CB, N], F8)
            xl_sb = cp.tile([128, CB, N], F8)
            gwh_sb = cp.tile([128, CB, E], F8)
            gwl_sb = cp.tile([128, CB, E], F8)
            gb_sb = cp.tile([E, 1], F32)
            ones_sb = cp.tile([E, EL], F16)
            b1_sb = cp.tile([128, EL, HB], F32)
            b2P_sb = cp.tile([128, EL, CB], F32)
            expT_sb = cp.tile([E, N], F32)
            expT16_sb = cp.tile([E, N], F16)
            g_bcast_sb = cp.tile([128, EL, N], F16)
            g_localT_sb = cp.tile([EL, N], F16)

            # DMA issue order = arrival order. The specially-traced first mm1
            # block needs, per cc-pair group: w1h slice -> xh half -> w1l
            # slice -> xl half (the PE's slot order is M, C1, C2 per ccp, so
            # xl arrives third). b1 (first gelu ~8us in) after the first
            # group; gate consts + the hbg1 w1 prefetch ride before the bulk
            # x second halves.
            xh_ap = xh_d[:, :].rearrange("(cc p) t -> p cc t", p=128)
            xl_ap = xl_d[:, :].rearrange("(cc p) t -> p cc t", p=128)
            w1h_first = sp.tile([128, CB, 512], F8, tag="w1", bufs=6, name="w1t")
            w1l_first = sp.tile([128, CB, 512], F8, tag="w1", bufs=6, name="w1t")
            for ccp in range(CP):
                cs = slice(2 * ccp, 2 * ccp + 2)
                nc.sync.dma_start(w1h_first[:, cs, :], w1ap(w1h_d, 0, 0)[:, cs, :])
                nc.sync.dma_start(xh_sb[:, cs, 0:TG], xh_ap[:, cs, 0:TG])
                nc.sync.dma_start(w1l_first[:, cs, :], w1ap(w1l_d, 0, 0)[:, cs, :])
                nc.sync.dma_start(xl_sb[:, cs, 0:TG], xl_ap[:, cs, 0:TG])
                if ccp == 0:
                    nc.sync.dma_start(b1_sb, b1_d[:, :, :])
            nc.sync.dma_start(gwh_sb, gwh_d[:, :].rearrange("(cc p) e -> p cc e", p=128))
            nc.sync.dma_start(gwl_sb, gwl_d[:, :].rearrange("(cc p) e -> p cc e", p=128))
            nc.sync.dma_start(gb_sb, gb_d[:, :])
            nc.sync.dma_start(ones_sb, ones_d[:, :])
            # prefetch e0's hbg1 w1 tiles so mm1 isn't gated on the bulk
            # second-half x transfers in front of it
            w1h_second = sp.tile([128, CB, 512], F8, tag="w1", bufs=6, name="w1t")
            w1l_second = sp.tile([128, CB, 512], F8, tag="w1", bufs=6, name="w1t")
            nc.sync.dma_start(w1h_second, w1ap(w1h_d, 0, 1))
            nc.sync.dma_start(w1l_second, w1ap(w1l_d, 0, 1))
            nc.sync.dma_start(xh_sb[:, :, TG:N], xh_ap[:, :, TG:N])
            nc.sync.dma_start(xl_sb[:, :, TG:N], xl_ap[:, :, TG:N])
            nc.sync.dma_start(b2P_sb, b2P_d[:, :, :])

            def dr3(ps, wh, wl, xh, xl, ccp, nccp):
                # one cc-pair's three hi/lo-cross slots into psum `ps`
                first = ccp == 0
                last = ccp == nccp - 1
                nc.tensor.matmul(ps, wh, xh, start=first, stop=False, perf_mode=DR)
                nc.tensor.matmul(ps, wh, xl, start=False, stop=False, perf_mode=DR)
                nc.tensor.matmul(ps, wl, xh, start=False, stop=last, perf_mode=DR)

            def emit_softmax():
                # gate logits via the same 3-slot fp8 path (gate noise feeds
                # the output at full weight); exp on ACT with the 1/4096
                # psum scale folded in; fp16 ones-matmul denominators; DVE
                # reciprocal; DRAM-bounce partition broadcast.
                # lg/dn borrow tag-"y" psum slots (mm2 needs them ~95us in;
                # these drain by ~20us).
                lgs = [
                    pp.tile([128, 512], F32, tag="y", bufs=4, name="lg")
                    for _ in range(4)
                ]
                for t4 in range(N // 512):
                    ts = slice(t4 * 512, (t4 + 1) * 512)
                    for ccp in range(CP):
                        cs = slice(2 * ccp, 2 * ccp + 2)
                        dr3(
                            lgs[t4][0:E, :],
                            gwh_sb[:, cs, :],
                            gwl_sb[:, cs, :],
                            xh_sb[:, cs, ts],
                            xl_sb[:, cs, ts],
                            ccp,
                            CP,
                        )
                for t4 in range(N // 512):
                    ts = slice(t4 * 512, (t4 + 1) * 512)
                    nc.scalar.activation(
                        expT_sb[:, ts], lgs[t4][0:E, :], AF.Exp, bias=gb_sb,
                        scale=1.0 / PSC1,
                    )
                    nc.scalar.activation(
                        expT16_sb[:, ts], lgs[t4][0:E, :], AF.Exp, bias=gb_sb,
                        scale=1.0 / PSC1,
                    )
                dns = [
                    pp.tile([128, 512], F32, tag="y", bufs=4, name="dn")
                    for _ in range(4)
                ]
                for t4 in range(N // 512):
                    ts = slice(t4 * 512, (t4 + 1) * 512)
                    nc.tensor.matmul(
                        dns[t4][0:EL, :], ones_sb[:, :], expT16_sb[:, ts],
                        start=True, stop=True,
                    )
                for t4 in range(N // 512):
                    ts = slice(t4 * 512, (t4 + 1) * 512)
                    rc = sp.tile([EL, 512], F32, tag="recip", bufs=2, name="rc")
                    nc.vector.reciprocal(rc, dns[t4][0:EL, :])
                    nc.vector.tensor_mul(g_localT_sb[:, ts], expT_sb[0:EL, ts], rc)
                g_dram = dp.tile([EL, N], F16, name="g_dram")
                nc.sync.dma_start(g_dram, g_localT_sb[:, :])
                for j in range(EL):
                    nc.sync.dma_start(
                        g_bcast_sb[:, j, :],
                        g_dram[j : j + 1, :].to_broadcast((128, N)),
                    )

            def emit_gelu_hilo(e, hbg, hps, hg16, hgh, hgl):
                # psum -> fp16 gelu (ACT), hg_hi = fp8(8*hg16) (Pool),
                # hg_lo = fp8(8*hg16 - hg_hi) (DVE stt), all at (hbi, ti)
                # granularity so the last h-block's fp8 tiles are ready
                # ~2us after its psum stops (mm2 needs them ~3.5us in).
                for hbi in range(4):
                    hb = hbg * 4 + hbi
                    for ti in range(TI):
                        lts = slice(ti * 512, (ti + 1) * 512)
                        nc.scalar.activation(
                            hg16[:, hbi, lts],
                            hps[hbi][ti],
                            act,
                            bias=b1_sb[:, e, hb : hb + 1],
                            scale=1.0 / PSC1,
                        )
                        nc.gpsimd.tensor_scalar_mul(
                            hgh[:, hb, lts], hg16[:, hbi, lts], SH
                        )
                        nc.vector.scalar_tensor_tensor(
                            out=hgl[:, hb, lts],
                            in0=hg16[:, hbi, lts],
                            scalar=SH,
                            in1=hgh[:, hb, lts],
                            op0=ALU.mult,
                            op1=ALU.subtract,
                        )

            def emit_mm1_first(hgh, hgl):
                # first h-block-group of (tg0, e0), traced ccp-OUTER across
                # all 8 psum banks: PE consumes each arriving fp8 chunk
                # immediately instead of stalling on the tail of one
                # accumulation group.
                hps8 = [
                    [
                        pp.tile(
                            [128, 512], F32,
                            tag=("h" if hbi < 2 else "y"),
                            bufs=4, name="hps",
                        )
                        for _ in range(TI)
                    ]
                    for hbi in range(4)
                ]
                for ccp in range(CP):
                    cs = slice(2 * ccp, 2 * ccp + 2)
                    for si, (wt, xt) in enumerate(
                        [(w1h_first, xh_sb), (w1h_first, xl_sb), (w1l_first, xh_sb)]
                    ):
                        for hbi in range(4):
                            for ti in range(TI):
                                nc.tensor.matmul(
                                    hps8[hbi][ti],
                                    wt[:, cs, hbi * 128 : (hbi + 1) * 128],
                                    xt[:, cs, ti * 512 : (ti + 1) * 512],
                                    start=(ccp == 0 and si == 0),
                                    stop=(ccp == CP - 1 and si == 2),
                                    perf_mode=DR,
                                )
                hg16 = sp.tile([128, 4, TG], F16, tag="hg16", bufs=2, name="hg16")
                emit_gelu_hilo(0, 0, hps8, hg16, hgh, hgl)

            def emit_mm1(tg, e, hgh, hgl, hbg_start=0, hbg_end=HB // 4):
                for hbg in range(hbg_start, hbg_end):
                    if tg == 0 and e == 0 and hbg == 1:
                        w1h_t, w1l_t = w1h_second, w1l_second
                    else:
                        w1h_t = sp.tile([128, CB, 512], F8, tag="w1", bufs=6, name="w1t")
                        w1l_t = sp.tile([128, CB, 512], F8, tag="w1", bufs=6, name="w1t")
                        nc.sync.dma_start(w1h_t, w1ap(w1h_d, e, hbg))
                        nc.sync.dma_start(w1l_t, w1ap(w1l_d, e, hbg))
                    hps = [
                        [
                            pp.tile([128, 512], F32, tag="h", bufs=4, name="hps")
                            for _ in range(TI)
                        ]
                        for _ in range(4)
                    ]
                    for hbi in range(4):
                        for ti in range(TI):
                            gts = slice(tg * TG + ti * 512, tg * TG + (ti + 1) * 512)
                            for ccp in range(CP):
                                cs = slice(2 * ccp, 2 * ccp + 2)
                                dr3(
                                    hps[hbi][ti],
                                    w1h_t[:, cs, hbi * 128 : (hbi + 1) * 128],
                                    w1l_t[:, cs, hbi * 128 : (hbi + 1) * 128],
                                    xh_sb[:, cs, gts],
                                    xl_sb[:, cs, gts],
                                    ccp,
                                    CP,
                                )
                    hg16 = sp.tile([128, 4, TG], F16, tag="hg16", bufs=2, name="hg16")
                    emit_gelu_hilo(e, hbg, hps, hg16, hgh, hgl)

            def emit_mm2(tg, e, hgh, hgl, yac):
                # psum[c,t] accumulates 12 h-pairs x 3 slots; the gate weight
                # and the (pre-scaled) b2 term apply on DVE into yac.
                for cb in range(CB):
                    w2h_t = sp.tile([128, HB, 128], F8, tag="w2", bufs=6, name="w2t")
                    w2l_t = sp.tile([128, HB, 128], F8, tag="w2", bufs=6, name="w2t")
                    nc.sync.dma_start(w2h_t, w2ap(w2h_d, e, cb))
                    nc.sync.dma_start(w2l_t, w2ap(w2l_d, e, cb))
                    for ti in range(TI):
                        lts = slice(ti * 512, (ti + 1) * 512)
                        gts = slice(tg * TG + ti * 512, tg * TG + (ti + 1) * 512)
                        yps = pp.tile([128, 512], F32, tag="y", bufs=4, name="yps")
                        for hp in range(HP):
                            hs = slice(2 * hp, 2 * hp + 2)
                            first = hp == 0
                            last = hp == HP - 1
                            nc.tensor.matmul(
                                yps, w2h_t[:, hs, :], hgh[:, hs, lts],
                                start=first, stop=False, perf_mode=DR,
                            )
                            nc.tensor.matmul(
                                yps, w2h_t[:, hs, :], hgl[:, hs, lts],
                                start=False, stop=False, perf_mode=DR,
                            )
                            nc.tensor.matmul(
                                yps, w2l_t[:, hs, :], hgh[:, hs, lts],
                                start=False, stop=last, perf_mode=DR,
                            )
                        if e == 0:
                            nc.vector.tensor_mul(
                                yac[:, cb, lts], g_bcast_sb[:, 0, gts], yps
                            )
                            for j in range(EL):
                                # b2 gate-weighted term (b2 pre-scaled x4096)
                                nc.vector.scalar_tensor_tensor(
                                    out=yac[:, cb, lts],
                                    in0=g_bcast_sb[:, j, gts],
                                    scalar=b2P_sb[:, j, cb : cb + 1],
                                    in1=yac[:, cb, lts],
                                    op0=ALU.mult,
                                    op1=ALU.add,
                                )
                        else:
                            ytmp = sp.tile([128, 512], F32, tag="ytmp", bufs=2,
                                           name="ytmp")
                            nc.vector.tensor_mul(
                                ytmp, g_bcast_sb[:, e, gts], yps
                            )
                            nc.vector.tensor_add(
                                yac[:, cb, lts], ytmp, yac[:, cb, lts]
                            )
                        if e == EL - 1:
                            # yac[cb] final: stream it out now (shortens tail)
                            nc.sync.dma_start(
                                outT_d[cb * 128 : (cb + 1) * 128, gts],
                                yac[:, cb, lts],
                            )

            # --- main. Trace order = PE order: the special first block
            # (fills the fp8 x/w arrival window), the gate prologue (drains
            # on ACT/DVE under the matmul stream), then the expert stream.
            for tg in range(TCG):
                hgh = sp.tile([128, HB, TG], F8, tag="hgh", bufs=1, name="hgh")
                hgl = sp.tile([128, HB, TG], F8, tag="hgl", bufs=1, name="hgl")
                yac = sp.tile([128, CB, TG], F32, tag="yacc", bufs=1, name="yac")
                for e in range(EL):
                    if tg == 0 and e == 0:
                        # softmax traced after hbg2 so its logits don't stall
                        # on the bulk second-half x DMAs
                        emit_mm1_first(hgh, hgl)
                        emit_mm1(tg, e, hgh, hgl, hbg_start=1, hbg_end=3)
                        emit_softmax()
                        emit_mm1(tg, e, hgh, hgl, hbg_start=3)
                    else:
                        emit_mm1(tg, e, hgh, hgl)
                    emit_mm2(tg, e, hgh, hgl, yac)

    nc.compile()
    return nc


def _get_nc():
    global _CACHED_NC
    if _CACHED_NC is None:
        _CACHED_NC = build_nc()
    return _CACHED_NC


def _hilo(a, scale):
    hi = (a * scale).astype(NP8)
    lo = (a * scale - hi.astype(np.float32)).astype(NP8)
    return hi, lo


def make_in_maps(x, gate_w, gate_b, w1, b1, w2, b2):
    x = np.asarray(x, np.float32)
    gate_w = np.asarray(gate_w, np.float32)
    gate_b = np.asarray(gate_b, np.float32)
    w1 = np.asarray(w1, np.float32)
    b1 = np.asarray(b1, np.float32)
    w2 = np.asarray(w2, np.float32)
    b2 = np.asarray(b2, np.float32)

    xT = np.ascontiguousarray(x.reshape(N, C).T)
    xh, xl = _hilo(xT, SX)
    w1h, w1l = _hilo(w1, SW1)
    w2h, w2l = _hilo(w2, SW2)

    ones32 = np.ones((E, EL), np.float16)

    in_maps = []
    for i in range(NCORES):
        lo_, hi_ = EL * i, EL * (i + 1)
        perm = list(range(lo_, hi_)) + [e for e in range(E) if not (lo_ <= e < hi_)]
        gwp = np.ascontiguousarray(gate_w[:, perm])
        gwh, gwl = _hilo(gwp, SW1)
        in_maps.append(
            {
                "xh": xh,
                "xl": xl,
                "gwh": gwh,
                "gwl": gwl,
                "gb": np.ascontiguousarray(gate_b[perm]).reshape(E, 1),
                "ones32": ones32,
                "w1h": w1h[lo_:hi_],
                "w1l": w1l[lo_:hi_],
                "b1": np.ascontiguousarray(
                    b1[lo_:hi_].reshape(EL, HB, 128).transpose(2, 0, 1)
                ),
                "w2h": w2h[lo_:hi_],
                "w2l": w2l[lo_:hi_],
                "b2P": np.ascontiguousarray(
                    b2[lo_:hi_].reshape(EL, CB, 128).transpose(2, 0, 1)
                )
                * PSC2,
            }
        )
    return in_maps


def kernel(x, gate_w, gate_b, w1, b1, w2, b2, _trace=False, _tmpdir=None):
    nc = _get_nc()
    in_maps = make_in_maps(x, gate_w, gate_b, w1, b1, w2, b2)
    res = run_bass_kernel_spmd(
        nc,
        in_maps,
        core_ids=list(range(NCORES)),
        trace=_trace,
        tmpdir=_tmpdir,
    )
    acc = res.results[0]["outT"].astype(np.float64)
    for r in res.results[1:]:
        acc += r["outT"]
    out = (acc / PSC2).T.reshape(B, T, C).astype(np.float32)
    if _trace:
        kernel._last_results = res
    return out
